# revision 15
# baseline (speedup 1.0000x reference)
"""Multi-head attention (b=8, n=1024, d=768, h=12) on 8 Trainium2 NeuronCores.

Strategy: pure data parallelism over the batch — core i computes batch element
i end-to-end (no collectives). Inside each core the computation is laid out
"feature-major" so no on-chip transposes are ever needed (see kernel_v1 for
the original derivation): q^T/k^T come out of the PE feature-major, V token-
major with a ones column so row 64 of each PV accumulator is the softmax
denominator; softmax is unnormalized exp with the 1/8 scale folded in and a
per-query reciprocal multiply after PV.

This version restructures the steady state around the real bottleneck, the
ACT exp stream (each [128,1024] exp is ~1.1us and the S-psum pool only has 2
slots, so the pair period was 2 exps/slot = 2.2us serialized on ACT):

  - exps are SPLIT between ACT (exact, Exp activation) and DVE (Schraudolph:
    bf16(exp(x)) bits ~= rint(A*x + B) computed as one tensor_scalar
    mult+add into an int16 view of the pt tile, then bitcast back to bf16
    for the PV matmul; measured-exact RNE on HW). 5/16 tiles per pair go to
    DVE, keeping rel_l2 ~9.6e-3 (threshold 2e-2).
  - the normalize chain (drain -> DMA hop -> recip -> broadcast -> mul) is
    a 3-stage software pipeline spread across slots 3/5/7 and the next
    pair's slots 1/3, so no engine FIFO head-of-lines on a cross-engine
    wait; chain tiles have bufs=4 so a lagging chain never back-pressures
    the PV psum rotation. GPSIMD runs ONLY partition_broadcast (mixing
    gpsimd op types costs ~5us per switch on HW); the muls run on DVE
    well after their broadcast.
  - next-pair q/k tiles finish accumulating at slots 3/6 so the next
    pair's S matmuls never wait on the qk drain.
  - input (x_k, wqk_k) pairs round-robin the three DGE queues (sync/
    scalar/gpsimd) on different queues so each contraction step's tiles
    stream in parallel; first matmul starts ~11us in.
  - the last pair's PV(h0) runs inside pair 5's slots (its qk-slots are
    free) and the projection for two token tiles accumulates k=0..4 while
    the final normalize chain completes, consuming attn[5] last.
"""

import sys

sys.path.insert(0, "/opt/trn_rl_repo")

import os

import numpy as np
import ml_dtypes

import concourse.bass as bass  # noqa: F401  (import keeps bass registered)
import concourse.mybir as mybir
import concourse.tile as tile
from concourse import bacc
from concourse.bass_utils import run_bass_kernel_spmd

N_CORES = 8
B, N, D = 8, 1024, 768
H, HD = 12, 64
SCALE = HD**-0.5
P = 128
KT = D // P  # 6 contraction tiles over d_model
MQ = 2 * D // P  # 12 output tiles over q+k features
TT = N // P  # 8 token tiles
F32 = mybir.dt.float32
I16 = mybir.dt.int16

# Matmul-operand dtype: "bf16" (bfloat16 operands, fp32 accumulate) or
# "f32r" (fp32 data run through the PE's fast fp32 mode).
DT_MODE = os.environ.get("ATTN_DT", "bf16")

# Schraudolph exp-on-DVE: bf16 bitpattern of exp(s/8) ~= rint(A*s + B).
EXP_A = float(128.0 * 0.125 * np.log2(np.e))
EXP_B = float(127.0 * 128.0 - 5.0)
# slots whose h1 exp runs on DVE (6 of 8; slots 0/7 stay on ACT so the
# close_pv chain work emitted at slot 7 lands in a DVE-exp-free slot; the
# slot-3 chain drain runs on ACT so DVE has room for an exp there.
# Interleaved A/B on HW: 6 DVE slots beats 5 by ~2us/run)
DVE_EXP_SLOTS = frozenset({1, 2, 3, 4, 5, 6})

# Only sync/scalar/gpsimd have DGE queues. (x_k, wqk_k) pairs round-robin
# over the three queues so the contraction tiles land in natural order.
K_ORDER = (0, 1, 2, 3, 4, 5)

# wqk columns are permuted on the host into consumption order
# [m0|m6|m1|m7|m2|m8|...] so the pair-0-critical chunk (m0,m6) of each
# k-tile is a single contiguous 2*P-column DMA that lands first.
M_ORDER = [m for i in range(KT) for m in (i, KT + i)]
MPOS = {m: i for i, m in enumerate(M_ORDER)}


def _np_mm_dtype():
    return ml_dtypes.bfloat16 if DT_MODE == "bf16" else np.float32


def _mm_dt():
    return mybir.dt.bfloat16 if DT_MODE == "bf16" else mybir.dt.float32


def build():
    nc = bacc.Bacc("TRN2", target_bir_lowering=False, debug=False)
    dt = _mm_dt()
    Exp = mybir.ActivationFunctionType.Exp

    def _mm(ap):
        if DT_MODE == "f32r":
            return ap.bitcast(mybir.dt.float32r)
        return ap

    xT_d = nc.dram_tensor("xT", [D, N], dt, kind="ExternalInput")
    wqkT_d = nc.dram_tensor("wqkT", [D, 2 * D], dt, kind="ExternalInput")
    wv_d = nc.dram_tensor("wv", [P, KT * D], dt, kind="ExternalInput")
    wp_d = nc.dram_tensor("wp", [P, KT * D], dt, kind="ExternalInput")
    qb_d = nc.dram_tensor("qb", [P, KT], F32, kind="ExternalInput")
    beff_d = nc.dram_tensor("beff", [P, D], F32, kind="ExternalInput")
    out_d = nc.dram_tensor("out", [N, D], dt, kind="ExternalOutput")

    with tile.TileContext(nc) as tc:
        with (
            tc.tile_pool(name="psum", bufs=1, space="PSUM") as psum,
            tc.tile_pool(name="persist", bufs=1) as persist,
            tc.tile_pool(name="work", bufs=1) as work,
        ):
            # ---- input DMAs, demand-ordered -----------------------------
            # Pair-0 only needs x_k plus the (m0,m6) "crit" columns of each
            # wqk_k (the host permutes wqk columns into consumption order),
            # so those stream first across all three queues; wv rides just
            # behind (needed when the phase-A V/S loop starts), then the
            # remaining wqk columns (restA = m1,m7,m2,m8 for qk(1)/qk(7)
            # and pair-1; restB = the rest). wp/beff are deferred to the
            # steady state (emitted at pair 2) — they're only consumed by
            # the projection epilogue and would otherwise steal HBM
            # bandwidth from the critical startup window.
            x_sb, wqk_sb = [], []
            for k in range(KT):
                xk = persist.tile([P, N], dt, tag=f"x{k}", name=f"x{k}")
                x_sb.append(xk)
                wqkk = persist.tile([P, 2 * D], dt, tag=f"wqk{k}", name=f"wqk{k}")
                wqk_sb.append(wqkk)
            wvA = persist.tile([P, KT * D], dt, tag="wv", name="wv")
            qbA = persist.tile([P, KT], F32, tag="qb", name="qb")
            wpA = work.tile([P, KT * D], dt, tag="wp", name="wp")
            bb = work.tile([P, D], F32, tag="bb", name="bb")

            def dma_x(q, k):
                q.dma_start(x_sb[k][:], xT_d.ap()[k * P : (k + 1) * P, :])

            def dma_w(q, k, c0, c1):
                q.dma_start(
                    wqk_sb[k][:, c0 * P : c1 * P],
                    wqkT_d.ap()[k * P : (k + 1) * P, c0 * P : c1 * P],
                )

            def dma_wv(q, k0, k1):
                q.dma_start(
                    wvA[:, k0 * D : k1 * D], wv_d.ap()[:, k0 * D : k1 * D]
                )

            CA, CB = 6, 12  # crit = cols [0:2), restA = [2:6), restB = [6:12)
            dma_x(nc.sync, 0); dma_w(nc.scalar, 0, 0, 2)
            dma_w(nc.sync, 1, 0, 2); dma_x(nc.scalar, 1)
            dma_x(nc.gpsimd, 2); dma_w(nc.gpsimd, 2, 0, 2)
            dma_x(nc.sync, 3); dma_w(nc.scalar, 3, 0, 2)
            dma_w(nc.sync, 4, 0, 2); dma_x(nc.scalar, 4)
            dma_x(nc.gpsimd, 5); dma_w(nc.gpsimd, 5, 0, 2)
            nc.sync.dma_start(qbA[:], qb_d.ap())
            dma_wv(nc.scalar, 0, 2)
            dma_wv(nc.sync, 2, 4)
            dma_wv(nc.gpsimd, 4, 6)
            dma_w(nc.scalar, 0, 2, CA); dma_w(nc.sync, 1, 2, CA)
            dma_w(nc.gpsimd, 2, 2, CA); dma_w(nc.scalar, 3, 2, CA)
            dma_w(nc.sync, 4, 2, CA); dma_w(nc.gpsimd, 5, 2, CA)
            dma_w(nc.scalar, 0, CA, CB); dma_w(nc.sync, 1, CA, CB)
            dma_w(nc.gpsimd, 2, CA, CB); dma_w(nc.scalar, 3, CA, CB)
            dma_w(nc.sync, 4, CA, CB); dma_w(nc.gpsimd, 5, CA, CB)

            # ---- emitters ------------------------------------------------
            v_sb = [None] * TT
            v_eng = [0] * TT

            def emit_v(t):
                vt = persist.tile([P, H, HD + 1], dt, tag=f"v{t}", name=f"v{t}")
                nc.vector.memset(vt[:, :, HD], 1.0)
                ps = psum.tile([P, N], F32, tag="mm2", bufs=2, name=f"vps{t}")
                for k in range(KT):
                    lhsT = _mm(x_sb[k])[:, t * P : (t + 1) * P]
                    nc.tensor.matmul(
                        ps[:, 0:512], lhsT, _mm(wvA)[:, k * D : k * D + 512],
                        start=(k == 0), stop=(k == KT - 1),
                    )
                    nc.tensor.matmul(
                        ps[:, 512:768], lhsT, _mm(wvA)[:, k * D + 512 : k * D + 768],
                        start=(k == 0), stop=(k == KT - 1),
                    )
                src = ps[:, 0:768].rearrange("p (h d) -> p h d", d=HD)
                # alternate the drain between ACT and DVE (psum readers)
                if t % 2 == 0:
                    nc.scalar.copy(vt[:, :, 0:HD], src)
                else:
                    nc.vector.tensor_copy(vt[:, :, 0:HD], src)
                v_sb[t] = vt

            qk_sb = [None] * MQ

            def open_qk(m):
                return psum.tile([P, N], F32, tag="mm2", bufs=2, name=f"qkps{m}")

            def qk_step(m, ps, k):
                mp = MPOS[m]
                lhsT = _mm(wqk_sb[k])[:, mp * P : (mp + 1) * P]
                for half in range(2):
                    nc.tensor.matmul(
                        ps[:, half * 512 : (half + 1) * 512],
                        lhsT,
                        _mm(x_sb[k])[:, half * 512 : (half + 1) * 512],
                        start=(k == 0), stop=(k == KT - 1),
                    )

            def close_qk(m, ps):
                qkm = persist.tile([P, N], dt, tag=f"qk{m}", name=f"qk{m}")
                if m < KT:
                    # q tile: add q_bias (per-partition scalar) on DVE
                    nc.vector.tensor_scalar_add(qkm[:], ps[:], qbA[:, m : m + 1])
                else:
                    # k tile: plain drain on ACT
                    nc.scalar.copy(qkm[:], ps[:])
                qk_sb[m] = qkm

            def emit_qk(m):
                ps = open_qk(m)
                for k in range(KT):
                    qk_step(m, ps, k)
                close_qk(m, ps)

            attn_sb = [
                persist.tile([P, N], dt, tag=f"attn{g}", name=f"attn{g}")
                for g in range(KT)
            ]
            pt_tiles = {}  # (g, j, hh) -> exp'd S^T tile, consumed by emit_pv

            def emit_sx(g, j):
                """S^T matmuls + exp for pair g, key tile j (both heads)."""
                q_t, k_t = qk_sb[g], qk_sb[KT + g]
                sps = []
                for hh in range(2):
                    sps.append(
                        psum.tile([P, N], F32, tag="sp", bufs=2, name=f"sp{g}_{j}_{hh}")
                    )
                # the two heads' K=64 matmuls occupy disjoint row halves of
                # the PE array and stream concurrently
                for half in range(2):
                    cols = slice(half * 512, (half + 1) * 512)
                    for hh in range(2):
                        lhsT = _mm(k_t)[hh * HD : (hh + 1) * HD, j * P : (j + 1) * P]
                        rhs = _mm(q_t)[hh * HD : (hh + 1) * HD, :]
                        nc.tensor.matmul(
                            sps[hh][:, cols], lhsT, rhs[:, cols],
                            tile_position=(hh * HD, 0),
                        )
                for hh in range(2):
                    pt = work.tile(
                        [P, N], dt, tag="pt", bufs=20, name=f"pt{g}_{j}_{hh}"
                    )
                    if hh == 1 and j in DVE_EXP_SLOTS:
                        # Schraudolph exp on DVE: int16 bits of bf16 exp
                        nc.vector.tensor_scalar(
                            pt[:].bitcast(I16), sps[hh][:],
                            EXP_A, EXP_B,
                            mybir.AluOpType.mult, mybir.AluOpType.add,
                        )
                    else:
                        # exact exp(S/8) on ACT straight out of PSUM
                        nc.scalar.activation(pt[:], sps[hh][:], Exp, scale=SCALE)
                    pt_tiles[(g, j, hh)] = pt

            def open_pv(g, hh):
                return psum.tile(
                    [HD + 1, N], F32, tag="mm2", bufs=2, name=f"pv{g}_{hh}"
                )

            def pv_step(g, hh, pp, j):
                pt = pt_tiles.pop((g, j, hh))
                lhsT = _mm(v_sb[j])[:, 2 * g + hh, :]  # [128, 65] V|1
                nc.tensor.matmul(
                    pp[:, 0:512], lhsT, _mm(pt)[:, 0:512],
                    start=(j == 0), stop=(j == TT - 1),
                )
                nc.tensor.matmul(
                    pp[:, 512:1024], lhsT, _mm(pt)[:, 512:1024],
                    start=(j == 0), stop=(j == TT - 1),
                )

            # The normalize chain crosses engines (drain -> DMA hop ->
            # recip -> broadcast -> mul) as a 3-stage software pipeline:
            # each stage is emitted slots (or a pair) later than its
            # producer so no engine FIFO ever head-of-lines on a
            # cross-engine wait. GPSIMD runs ONLY partition_broadcast
            # (mixing gpsimd op types costs ~5us per switch on hardware);
            # the normalize muls are DVE, emitted well after the
            # broadcast. Chain tiles have bufs=4 so a lagging chain never
            # back-pressures the PV psum rotation.
            CHAIN_BUFS = 4
            _chain = {}

            def chain_drain(g, hh, pp, eng=None):
                # drain all 65 rows (row HD = denominator) at base 0; the
                # denominator row hops to partition 0 of dn via DMA
                sb = work.tile([HD + 1, N], F32, tag="sbh", bufs=CHAIN_BUFS,
                               name=f"sbh{hh}_{g}")
                if eng is None:
                    eng = "act" if hh == 0 else "dve"
                if eng == "act":
                    nc.scalar.copy(sb[:], pp[:])
                else:
                    nc.vector.tensor_copy(sb[:], pp[:])
                dn = work.tile([1, N], F32, tag="dn", bufs=CHAIN_BUFS,
                               name=f"dn{hh}_{g}")
                nc.sync.dma_start(dn[:], sb[HD : HD + 1, :])
                _chain[(g, hh)] = (sb, dn, None)

            def chain_recip(g, hh):
                sb, dn, _ = _chain[(g, hh)]
                nc.vector.reciprocal_approx_fast(dn[:], dn[:])
                rb = work.tile([HD, N], F32, tag="rb", bufs=CHAIN_BUFS,
                               name=f"rb{hh}_{g}")
                nc.gpsimd.partition_broadcast(rb[:], dn[:])
                _chain[(g, hh)] = (sb, dn, rb)

            def chain_mul(g, hh):
                sb, dn, rb = _chain.pop((g, hh))
                # normalize mul on DVE; for h1 the OUTPUT shifts 0->64
                # (only inputs must share a base partition)
                dst = attn_sb[g][0:HD, :] if hh == 0 else attn_sb[g][HD:P, :]
                nc.vector.tensor_mul(dst, sb[0:HD, :], rb[:])

            def close_pv(g, hh, pp):
                chain_drain(g, hh, pp)
                chain_recip(g, hh)
                chain_mul(g, hh)

            # ---- phase A: qkv in DMA-arrival order + V + pair-0 S/exp ----
            # pair 0's slots mirror the steady state, with V matmuls in
            # place of PV steps and qk(1)/qk(KT+1) interleaved like any
            # next-pair qk work.
            ps_q0 = open_qk(0)
            ps_k0 = open_qk(KT)
            for k in K_ORDER:
                qk_step(0, ps_q0, k)
                qk_step(KT, ps_k0, k)
            close_qk(0, ps_q0)
            close_qk(KT, ps_k0)
            for j in range(TT):
                emit_sx(0, j)
                emit_v(j)
            emit_qk(1)
            emit_qk(KT + 1)

            # ---- steady state: pairs 1..KT-1 ------------------------------
            # per slot: 2 pv_steps(g-1), next-pair qk matmuls, THEN S+exp(g)
            # last — so by the time the PE drains to slot j+1's S matmuls
            # (which wait on slot j's exps through the 2-buf sp rotation)
            # the exps have had a full slot of pv/qk streaming to complete.
            # q accumulates over slots 0-3 (close at 3), k over slots 4-6
            # (close at 6) so the next pair's S never waits on the drain.
            # Normalize chain schedule (chain for pair p = g-1 while pair
            # g runs; each stage lands after its producer's latency):
            #   slot 1: recip+bcast for (g-2, h1)   slot 3: mul (g-2, h1),
            #           drain (g-1, h0) + hop       slot 5: recip+bcast
            #           (g-1, h0)                   slot 7: mul (g-1, h0),
            #           drain (g-1, h1) + hop
            Q_CHUNKS = ([0, 1], [2], [3], [4, 5])
            K_CHUNKS = ([0, 1], [2, 3], [4, 5])
            pp50 = None
            for g in range(1, KT):
                nxt = g + 1 if g + 1 < KT else None
                pp = qkps = m = None
                for slot in range(TT):
                    hh = 0 if slot < 4 else 1
                    if slot in (0, 4):
                        pp = open_pv(g - 1, hh)
                        if nxt is not None:
                            m = nxt if slot == 0 else KT + nxt
                            qkps = open_qk(m)
                    pv_step(g - 1, hh, pp, 2 * (slot % 4))
                    pv_step(g - 1, hh, pp, 2 * (slot % 4) + 1)
                    if nxt is not None:
                        chunks = Q_CHUNKS if slot < 4 else K_CHUNKS
                        if slot % 4 < len(chunks):
                            for k in chunks[slot % 4]:
                                qk_step(m, qkps, k)
                    elif slot >= 4:
                        # pair 5's qk-slots are free: catch up PV(5, h0)
                        if slot == 4:
                            pp50 = open_pv(KT - 1, 0)
                        for j in (2 * (slot - 4), 2 * (slot - 4) + 1):
                            if j < 7:
                                pv_step(KT - 1, 0, pp50, j)
                    emit_sx(g, slot)
                    if g == 2 and slot == 0:
                        # deferred weight loads: the input queues are idle
                        # now and wp/beff aren't needed until the epilogue
                        nc.sync.dma_start(wpA[:], wp_d.ap())
                    if g == 2 and slot == 4:
                        nc.sync.dma_start(bb[:], beff_d.ap())
                    if nxt is not None and (slot == 3 or slot == 6):
                        close_qk(m, qkps)
                    if slot == 1 and g >= 2:
                        chain_recip(g - 2, 1)
                    if slot == 3:
                        if g >= 2:
                            chain_mul(g - 2, 1)
                        chain_drain(g - 1, 0, pp)
                    if slot == 5:
                        chain_recip(g - 1, 0)
                    if slot == 7:
                        chain_mul(g - 1, 0)
                        chain_drain(g - 1, 1, pp)

            # ---- epilogue: finish PV(5) + outstanding chains, overlap
            # proj with the chain latency ---------------------------------
            # Proj runs in two waves of 4 token tiles (psum limit: 2 "sp" +
            # 2 "mm2" rings free up once the last exps/PV drains complete).
            # All chain-independent matmuls (k=0..4) are emitted BEFORE any
            # k=5 step so the pair-5 normalize chains never head-of-line
            # block the PE. Output tiles are written bf16 and DMA'd out
            # round-robin over the three DGE queues as soon as each token
            # tile's k=5 accumulation lands.
            def proj_steps(t, ps, ks):
                for k in ks:
                    lhsT = _mm(attn_sb[k])[:, t * P : (t + 1) * P]
                    nc.tensor.matmul(
                        ps[:, 0:512], lhsT, _mm(wpA)[:, k * D : k * D + 512],
                        start=(k == 0), stop=(k == KT - 1),
                    )
                    nc.tensor.matmul(
                        ps[:, 512:768], lhsT, _mm(wpA)[:, k * D + 512 : k * D + 768],
                        start=(k == 0), stop=(k == KT - 1),
                    )

            out_queues = (nc.sync, nc.scalar, nc.gpsimd)

            def close_proj(t, ps):
                ot = work.tile([P, D], dt, tag="ot", bufs=8, name=f"ot{t}")
                nc.vector.tensor_add(ot[:], ps[:, 0:768], bb[:])
                out_queues[t % 3].dma_start(
                    out_d.ap()[t * P : (t + 1) * P, :], ot[:]
                )

            def open_proj(t):
                tag = "sp" if t % 4 < 2 else "mm2"
                return psum.tile([P, N], F32, tag=tag, bufs=2, name=f"ops{t}")

            pv_step(KT - 1, 0, pp50, 7)
            chain_drain(KT - 1, 0, pp50)
            chain_recip(KT - 2, 1)
            pp51 = open_pv(KT - 1, 1)
            for j in range(2):
                pv_step(KT - 1, 1, pp51, j)
            chain_mul(KT - 2, 1)
            for j in range(2, 4):
                pv_step(KT - 1, 1, pp51, j)
            chain_recip(KT - 1, 0)
            for j in range(4, TT):
                pv_step(KT - 1, 1, pp51, j)
            chain_drain(KT - 1, 1, pp51, eng="act")
            chain_mul(KT - 1, 0)
            # wave 1: t=0..3 prefill k=0..3 while the (5,*) chains complete
            ps_t = {}
            for t in range(4):
                ps_t[t] = open_proj(t)
                proj_steps(t, ps_t[t], range(4))
                if t == 1:
                    chain_recip(KT - 1, 1)
                if t == 2:
                    chain_mul(KT - 1, 1)
            for t in range(4):
                proj_steps(t, ps_t[t], [4])
            for t in range(4):
                proj_steps(t, ps_t[t], [5])
                close_proj(t, ps_t[t])
            # wave 2: t=4..7 (psum rings freed by wave-1 closes)
            for t in range(4, TT):
                ps = open_proj(t)
                proj_steps(t, ps, range(KT))
                close_proj(t, ps)

    nc.compile()
    return nc


_NC_CACHE = None


def _get_nc():
    global _NC_CACHE
    if _NC_CACHE is None:
        _NC_CACHE = build()
    return _NC_CACHE


def make_in_maps(x, w_qkv, q_bias, v_bias, w_proj, b_proj):
    mmdt = _np_mm_dtype()
    wqkT = np.ascontiguousarray(w_qkv[: 2 * D].T).astype(mmdt)
    # permute columns into consumption order [m0|m6|m1|m7|...]
    wqkP = np.ascontiguousarray(
        np.concatenate([wqkT[:, m * P : (m + 1) * P] for m in M_ORDER], axis=1)
    )
    wvT = np.ascontiguousarray(w_qkv[2 * D :].T).astype(mmdt)
    wvALL = np.ascontiguousarray(
        wvT.reshape(KT, P, D).transpose(1, 0, 2).reshape(P, KT * D)
    )
    wpT = np.ascontiguousarray(w_proj.T).astype(mmdt)
    wpALL = np.ascontiguousarray(
        wpT.reshape(KT, P, D).transpose(1, 0, 2).reshape(P, KT * D)
    )
    qb = np.ascontiguousarray(q_bias.reshape(KT, P).T).astype(np.float32)
    beff_row = (
        w_proj.astype(np.float64) @ v_bias.astype(np.float64) + b_proj
    ).astype(np.float32)
    beff = np.ascontiguousarray(np.tile(beff_row, (P, 1)))
    shared = {"wqkT": wqkP, "wv": wvALL, "wp": wpALL, "qb": qb, "beff": beff}
    in_maps = []
    for i in range(N_CORES):
        m = dict(shared)
        m["xT"] = np.ascontiguousarray(x[i].T).astype(mmdt)
        in_maps.append(m)
    return in_maps


def kernel(x, w_qkv, q_bias, v_bias, w_proj, b_proj, _trace=False, _tmpdir=None):
    x = np.asarray(x)
    nc = _get_nc()
    in_maps = make_in_maps(
        np.asarray(x, dtype=np.float32),
        np.asarray(w_qkv, dtype=np.float32),
        np.asarray(q_bias, dtype=np.float32),
        np.asarray(v_bias, dtype=np.float32),
        np.asarray(w_proj, dtype=np.float32),
        np.asarray(b_proj, dtype=np.float32),
    )
    res = run_bass_kernel_spmd(
        nc, in_maps, core_ids=list(range(N_CORES)), trace=_trace, tmpdir=_tmpdir
    )
    out = np.stack([res.results[i]["out"] for i in range(N_CORES)], axis=0)
    if _trace:
        return out.astype(np.float32), res
    return out.astype(np.float32)

